# revision 20
# baseline (speedup 1.0000x reference)
"""TRN2 Bass kernel: relu + per-row top-32 masking for x [4096, 32768] f32.

kernel(x) -> (relu(x), topk_masked) matching:
    y = relu(x); vals, idx = top_k(y, 32); xz = zeros.at[rows, idx].set(vals)

Sharding: pure data parallel over rows, 8 NeuronCores x [512, 32768].

Per-core algorithm (exact for continuous random input):
  stream x in column sub-tiles: chunk maxes (1024 chunks of 32) on DVE
  read the RAW x tile (chunk max of x == chunk max of relu(x) whenever
  the chunk has any positive entry; an all-negative chunk is never a
  top-32 candidate); relu afterwards in place on ScalarE, y stored via
  the Act HWDGE ring so the SP load FIFO never waits on compute.
  Top-32 chunks via 4 rounds of DVE max8 + max_index (indices direct;
  f32 chunk maxes make within-round value ties measure-zero) +
  match_replace zap; indirect-DMA gather those 32 chunks/row from DRAM;
  max8 + match_replace rounds on the gathered [P, 1024] pick the top-32
  elements; masked chunks (Gf - zapped) are indirect-DMA scattered into
  the pre-zeroed xz output, so only 1/32 of xz is ever written.

Schedule (Tile list-scheduling, per-block priority tiers): stream tier
  first; selection tiers ordered G-stage(b-1)+scatters(b-1) < M/ext(b) <
  gathers(b) so Pool's in-order stream is [s_{b-1}, g_b] per window and
  never head-of-line blocks on late data; scatter->scatter WAW deps
  (provably disjoint chunks) are stripped.

Post-mortems:
  2026-08-07/08: Pool-bound 256 indirect DMAs x ~1.1us; SWDGE cost is
    ~994ns FIXED per call + 0.34ns/desc; multi-offset tables compiled but
    corrupted data on HW (re-verified 2026-08-09 via probe_multioffset /
    see `multi` flag); custom dma_gather ~7ns/desc - no better.
  2026-08-09 (627us harness baseline -> 534us): found three couplings:
    (1) selection for block b was emitted after block b+1's reduces in
    DVE's in-order stream -> 168us post-stream tail. (2) in-place relu
    made the y-store wait on DVE's reduce (WAR), so any DVE selection
    burst stalled the stream; fixed with a separate yt tile. (3) THE BIG
    ONE: Tile caps outstanding SWDGE DMAs at 8 (DMASW completion-sem
    lanes); each indirect call waits for the 8-back call's COMPLETION,
    which under stream load sits behind ~6.5us stream packets on the
    shared SDMA engines (30-50us tails). sub=2048 (8KB descriptors)
    halves packet drain time and restores Pool to ~1.1us/call. Window
    order [s_{b-1} half, g_b, s_{b-1} half] + G-stage at the END of its
    own window keeps gather completions a full window ahead of their
    DVE consumer. bf16 chunk-max reduce was tried and REVERTED: no DVE
    speedup, and quantized maxes drop true candidate chunks (positional
    err^2 ~ v^2 per swap -> rel 0.128). sub=1024 crashed the device
    (NRT_EXEC_UNIT_UNRECOVERABLE) - do not ship.
  Remaining structure at 534us: ~90us lead-in (block 0 stream before
    first gather), ~85-95us/window steady state, ~150us tail (last
    block's M+g+G+s chain is stream-end-gated; 64 indirect calls x 1.1us
    of it is irreducible fixed SWDGE overhead).
"""

import os
import sys

if "/opt/trn_rl_repo" not in sys.path:
    sys.path.insert(0, "/opt/trn_rl_repo")

import numpy as np

import concourse.bass as bass
import concourse.mybir as mybir
from concourse import bacc
from concourse.bass_utils import run_bass_kernel_spmd
from concourse.tile import TileContext

F32 = mybir.dt.float32
I32 = mybir.dt.int32
U32 = mybir.dt.uint32

N_ROWS = 4096
N_COLS = 32768
N_CORES = 8
K = 32           # top-k
P = 128          # rows per block (partitions)

LAST_EXEC_TIME_NS = None
LAST_TRACE_DIR = None
_CACHED = {}


def _set_prio(handles, prio):
    for h in handles:
        ins = getattr(h, "ins", h)
        if ins.bass_priority is not None:
            ins.bass_priority = prio


def _build(R: int, D: int, sub: int = 2048, g_bufs: int = 3, x_bufs: int = 12,
           y_bufs: int = 4, m_bufs: int = 2, s_bufs: int = 4, cl: int = 32,
           multi: int = 0, scratch: int = 32768):
    # sub=2048 keeps stream descriptors at 8KB/partition: indirect-DMA
    # completions (which gate Pool issue through the 8-deep DMASW
    # completion-sem lanes, max 8 outstanding SWDGE DMAs) then only wait
    # behind ~3us stream packets instead of ~6.5us ones. With sub=4096 the
    # lane ring capped Pool at ~0.6 indirect calls/us and the whole
    # selection pipeline slid into a post-stream tail.
    C = D // cl
    n_blocks = R // P
    n_sub = D // sub
    sub_chunks = sub // cl

    STREAM = -3_000_000
    SEL = -2_000_000   # + b*1000 + stage

    nc = bacc.Bacc("TRN2", target_bir_lowering=False, debug=False,
                   dynamic_dma_scratch_size=scratch)
    x = nc.declare_dram_parameter("x", [R, D], F32, isOutput=False)
    y = nc.declare_dram_parameter("y", [R, D], F32, isOutput=True)
    xz = nc.declare_dram_parameter("xz", [R, D], F32, isOutput=True)

    x_chunks = x[:].rearrange("r (c l) -> (r c) l", l=cl)
    xz_chunks = xz[:].rearrange("r (c l) -> (r c) l", l=cl)

    with TileContext(nc) as tc:
        with (
            tc.tile_pool(name="consts", bufs=1) as const_pool,
            tc.tile_pool(name="xstream", bufs=x_bufs) as x_pool,
            tc.tile_pool(name="ystream", bufs=y_bufs) as y_pool,
            tc.tile_pool(name="mstage", bufs=m_bufs) as m_pool,
            tc.tile_pool(name="gstage", bufs=g_bufs) as g_pool,
            tc.tile_pool(name="small", bufs=s_bufs) as s_pool,
        ):
            # rowbase_b[p, 0] = p*C + b*P*C  (global chunk id base per row)
            rowbases = []
            for b in range(n_blocks):
                rb_i = const_pool.tile([P, 1], I32, tag=f"rowbase_i{b}")
                nc.gpsimd.iota(rb_i[:], pattern=[[0, 1]], base=b * P * C,
                               channel_multiplier=C)
                rowbases.append(rb_i)

            scatter_names = set()
            pending = None  # (b, sel, G) awaiting G-stage + scatters

            def emit_gstage_and_scatter(state):
                sb, s_sel, G = state
                # G-stage at the END of block sb's own window (as soon as its
                # gathers land); scatters split across the NEXT window: 16
                # fill Pool's sel-wait before g_{sb+1}, 16 after it.
                g_tier = SEL + sb * 1000 + 250
                s_tier_a = SEL + (sb + 1) * 1000 + 100
                s_tier_b = SEL + (sb + 1) * 1000 + 400
                Gf = G[:].rearrange("p k l -> p (k l)")
                Gw = g_pool.tile([P, K * cl], F32, tag="Gw")
                gx8 = s_pool.tile([P, 8], F32, tag="gx8")
                gh = []
                gsrc = Gf
                for _ in range(K // 8):
                    gh.append(nc.vector.max(gx8[:], gsrc))
                    gh.append(nc.vector.match_replace(out=Gw[:], in_to_replace=gx8[:],
                                                      in_values=gsrc, imm_value=0.0))
                    gsrc = Gw[:]
                gh.append(nc.vector.tensor_tensor(out=Gw[:], in0=Gf, in1=Gw[:],
                                                  op=mybir.AluOpType.subtract))
                _set_prio(gh, g_tier)
                Gw3 = Gw[:].rearrange("p (k l) -> p k l", l=cl)
                new_ins = []
                if multi:
                    new_ins.append(nc.gpsimd.indirect_dma_start(
                        out=xz_chunks,
                        out_offset=bass.IndirectOffsetOnAxis(ap=s_sel[:, :], axis=0),
                        in_=Gw3[:, :, :],
                        in_offset=None,
                    ))
                else:
                    for k in range(K):
                        new_ins.append(nc.gpsimd.indirect_dma_start(
                            out=xz_chunks,
                            out_offset=bass.IndirectOffsetOnAxis(ap=s_sel[:, k:k + 1], axis=0),
                            in_=Gw3[:, k, :],
                            in_offset=None,
                        ))
                for ins in new_ins:
                    ins_ = getattr(ins, "ins", ins)
                    for dep in list(ins_.sync_dependency_names()):
                        if dep in scatter_names:
                            ins_.try_remove_dependency(dep)
                    scatter_names.add(ins_.name)
                _set_prio(new_ins[:len(new_ins) // 2], s_tier_a)
                _set_prio(new_ins[len(new_ins) // 2:], s_tier_b)

            for b in range(n_blocks):
                r0 = b * P
                M = m_pool.tile([P, C], F32, tag="M")
                stream_h = []
                for s in range(n_sub):
                    c0 = s * sub
                    xt = x_pool.tile([P, sub], F32, tag="xt")
                    h = [nc.sync.dma_start(out=xt[:], in_=x[r0:r0 + P, c0:c0 + sub])]
                    # chunk maxes from RAW x: decouples DVE from the relu
                    h.append(nc.vector.tensor_reduce(
                        out=M[:, s * sub_chunks:(s + 1) * sub_chunks],
                        in_=xt[:].rearrange("p (c l) -> p c l", l=cl),
                        axis=mybir.AxisListType.X,
                        op=mybir.AluOpType.max,
                    ))
                    # relu into a separate tile: the y stream (load -> relu ->
                    # store, all SP/Act) must never wait on DVE's reduce
                    yt = y_pool.tile([P, sub], F32, tag="yt")
                    h.append(nc.scalar.activation(yt[:], xt[:], mybir.ActivationFunctionType.Relu))
                    h.append(nc.scalar.dma_start(out=y[r0:r0 + P, c0:c0 + sub], in_=yt[:]))
                    # unique, monotonically increasing priorities keep the
                    # per-engine tie-break order deterministic
                    for j, hh in enumerate(h):
                        _set_prio([hh], STREAM + (b * n_sub + s) * 8 + j)
                    stream_h += h

                # previous block's G-stage + scatters (tiers computed inside
                # from the pending block's own index)
                if pending is not None:
                    emit_gstage_and_scatter(pending)

                # top-32 chunks: max8 + max_index + match_replace rounds
                m_h = []
                Mw = m_pool.tile([P, C], F32, tag="Mw")
                mx8 = s_pool.tile([P, 8], F32, tag="mx8")
                mi8 = s_pool.tile([P, 8], U32, tag="mi8")
                sel = s_pool.tile([P, K], I32, tag="offs")
                g_h = []
                G = g_pool.tile([P, K, cl], F32, tag="G")
                src = M
                for r in range(K // 8):
                    sl = slice(r * 8, (r + 1) * 8)
                    m_h.append(nc.vector.max(mx8[:], src[:]))
                    m_h.append(nc.vector.max_index(mi8[:], mx8[:], src[:]))
                    if r < K // 8 - 1:
                        m_h.append(nc.vector.match_replace(out=Mw[:], in_to_replace=mx8[:],
                                                           in_values=src[:], imm_value=-1.0))
                        src = Mw
                    # sel = chunk_idx + p*C + b*P*C
                    m_h.append(nc.vector.tensor_tensor(
                        out=sel[:, sl], in0=mi8[:],
                        in1=rowbases[b][:, :1].to_broadcast([P, 8]),
                        op=mybir.AluOpType.add))
                    if not multi:
                        for k in range(r * 8, (r + 1) * 8):
                            g_h.append(nc.gpsimd.indirect_dma_start(
                                out=G[:, k, :], out_offset=None,
                                in_=x_chunks,
                                in_offset=bass.IndirectOffsetOnAxis(ap=sel[:, k:k + 1], axis=0),
                            ))
                if multi:
                    g_h.append(nc.gpsimd.indirect_dma_start(
                        out=G[:, :, :], out_offset=None,
                        in_=x_chunks,
                        in_offset=bass.IndirectOffsetOnAxis(ap=sel[:, :], axis=0),
                    ))
                _set_prio(m_h, SEL + b * 1000)
                _set_prio(g_h, SEL + b * 1000 + 200)
                pending = (b, sel, G)
            if pending is not None:
                emit_gstage_and_scatter(pending)
    nc.finalize()
    return nc


def kernel(x: np.ndarray):
    global LAST_EXEC_TIME_NS, LAST_TRACE_DIR
    x = np.ascontiguousarray(np.asarray(x, dtype=np.float32))
    assert x.shape == (N_ROWS, N_COLS), x.shape
    Rs = N_ROWS // N_CORES

    cfg = {}
    env = os.environ.get("BASS_KCFG")
    if env:
        for kv in env.split(","):
            k, v = kv.split("=")
            cfg[k] = int(v)
    key = tuple(sorted(cfg.items()))
    if key not in _CACHED:
        _CACHED[key] = _build(Rs, N_COLS, **cfg)
    nc = _CACHED[key]

    in_maps = [{"x": x[i * Rs:(i + 1) * Rs]} for i in range(N_CORES)]
    tmpdir = None
    if os.environ.get("BASS_TRACE"):
        import tempfile
        tmpdir = tempfile.mkdtemp(prefix="topk_trace_")
        LAST_TRACE_DIR = tmpdir
    res = run_bass_kernel_spmd(nc, in_maps, core_ids=list(range(N_CORES)),
                               tmpdir=tmpdir)
    LAST_EXEC_TIME_NS = res.exec_time_ns

    y = np.concatenate([np.asarray(res.results[i]["y"]).reshape(Rs, N_COLS)
                        for i in range(N_CORES)], axis=0)
    xz = np.concatenate([np.asarray(res.results[i]["xz"]).reshape(Rs, N_COLS)
                         for i in range(N_CORES)], axis=0)
    return y, xz


# revision 21
# speedup vs baseline: 1.0680x; 1.0680x over previous
"""TRN2 Bass kernel: relu + per-row top-32 masking for x [4096, 32768] f32.

kernel(x) -> (relu(x), topk_masked) matching:
    y = relu(x); vals, idx = top_k(y, 32); xz = zeros.at[rows, idx].set(vals)

Sharding: pure data parallel over rows, 8 NeuronCores x [512, 32768].

Per-core algorithm (exact for continuous random input):
  stream x in column sub-tiles: chunk maxes (1024 chunks of 32) on DVE
  read the RAW x tile (chunk max of x == chunk max of relu(x) whenever
  the chunk has any positive entry; an all-negative chunk is never a
  top-32 candidate); relu afterwards in place on ScalarE, y stored via
  the Act HWDGE ring so the SP load FIFO never waits on compute.
  Top-32 chunks via 4 rounds of DVE max8 + max_index (indices direct;
  f32 chunk maxes make within-round value ties measure-zero) +
  match_replace zap; indirect-DMA gather those 32 chunks/row from DRAM;
  max8 + match_replace rounds on the gathered [P, 1024] pick the top-32
  elements; masked chunks (Gf - zapped) are indirect-DMA scattered into
  the pre-zeroed xz output, so only 1/32 of xz is ever written.

Schedule (Tile list-scheduling, per-block priority tiers): stream tier
  first; selection tiers ordered G-stage(b-1)+scatters(b-1) < M/ext(b) <
  gathers(b) so Pool's in-order stream is [s_{b-1}, g_b] per window and
  never head-of-line blocks on late data; scatter->scatter WAW deps
  (provably disjoint chunks) are stripped.

Post-mortems:
  2026-08-07/08: Pool-bound 256 indirect DMAs x ~1.1us; SWDGE cost is
    ~994ns FIXED per call + 0.34ns/desc; multi-offset tables compiled but
    corrupted data on HW (re-verified 2026-08-09 via probe_multioffset /
    see `multi` flag); custom dma_gather ~7ns/desc - no better.
  2026-08-09 (627us harness baseline -> 534us): found three couplings:
    (1) selection for block b was emitted after block b+1's reduces in
    DVE's in-order stream -> 168us post-stream tail. (2) in-place relu
    made the y-store wait on DVE's reduce (WAR), so any DVE selection
    burst stalled the stream; fixed with a separate yt tile. (3) THE BIG
    ONE: Tile caps outstanding SWDGE DMAs at 8 (DMASW completion-sem
    lanes); each indirect call waits for the 8-back call's COMPLETION,
    which under stream load sits behind ~6.5us stream packets on the
    shared SDMA engines (30-50us tails). sub=2048 (8KB descriptors)
    halves packet drain time and restores Pool to ~1.1us/call. Window
    order [s_{b-1} half, g_b, s_{b-1} half] + G-stage at the END of its
    own window keeps gather completions a full window ahead of their
    DVE consumer. bf16 chunk-max reduce was tried and REVERTED: no DVE
    speedup, and quantized maxes drop true candidate chunks (positional
    err^2 ~ v^2 per swap -> rel 0.128). sub=1024 crashed the device
    (NRT_EXEC_UNIT_UNRECOVERABLE) - do not ship.
  Remaining structure at 534us: ~90us lead-in (block 0 stream before
    first gather), ~85-95us/window steady state, ~150us tail (last
    block's M+g+G+s chain is stream-end-gated; 64 indirect calls x 1.1us
    of it is irreducible fixed SWDGE overhead).
"""

import os
import sys

if "/opt/trn_rl_repo" not in sys.path:
    sys.path.insert(0, "/opt/trn_rl_repo")

import numpy as np

import concourse.bass as bass
import concourse.mybir as mybir
from concourse import bacc
from concourse.bass_utils import run_bass_kernel_spmd
from concourse.tile import TileContext

F32 = mybir.dt.float32
I32 = mybir.dt.int32
U32 = mybir.dt.uint32

N_ROWS = 4096
N_COLS = 32768
N_CORES = 8
K = 32           # top-k
P = 128          # rows per block (partitions)

LAST_EXEC_TIME_NS = None
LAST_TRACE_DIR = None
_CACHED = {}


def _set_prio(handles, prio):
    for h in handles:
        ins = getattr(h, "ins", h)
        if ins.bass_priority is not None:
            ins.bass_priority = prio


def _build(R: int, D: int, sub: int = 2048, g_bufs: int = 3, x_bufs: int = 12,
           y_bufs: int = 4, m_bufs: int = 2, s_bufs: int = 4, cl: int = 32,
           multi: int = 0, scratch: int = 32768):
    # sub=2048 keeps stream descriptors at 8KB/partition: indirect-DMA
    # completions (which gate Pool issue through the 8-deep DMASW
    # completion-sem lanes, max 8 outstanding SWDGE DMAs) then only wait
    # behind ~3us stream packets instead of ~6.5us ones. With sub=4096 the
    # lane ring capped Pool at ~0.6 indirect calls/us and the whole
    # selection pipeline slid into a post-stream tail.
    C = D // cl
    n_blocks = R // P
    n_sub = D // sub
    sub_chunks = sub // cl

    STREAM = -3_000_000
    SEL = -2_000_000   # + b*1000 + stage

    nc = bacc.Bacc("TRN2", target_bir_lowering=False, debug=False,
                   dynamic_dma_scratch_size=scratch)
    x = nc.declare_dram_parameter("x", [R, D], F32, isOutput=False)
    y = nc.declare_dram_parameter("y", [R, D], F32, isOutput=True)
    xz = nc.declare_dram_parameter("xz", [R, D], F32, isOutput=True)

    x_chunks = x[:].rearrange("r (c l) -> (r c) l", l=cl)
    xz_chunks = xz[:].rearrange("r (c l) -> (r c) l", l=cl)

    with TileContext(nc) as tc:
        with (
            tc.tile_pool(name="consts", bufs=1) as const_pool,
            tc.tile_pool(name="xstream", bufs=x_bufs) as x_pool,
            tc.tile_pool(name="ystream", bufs=y_bufs) as y_pool,
            tc.tile_pool(name="mstage", bufs=m_bufs) as m_pool,
            tc.tile_pool(name="gstage", bufs=g_bufs) as g_pool,
            tc.tile_pool(name="small", bufs=s_bufs) as s_pool,
        ):
            # rowbase_b[p, 0] = p*C + b*P*C  (global chunk id base per row)
            rowbases = []
            for b in range(n_blocks):
                rb_i = const_pool.tile([P, 1], I32, tag=f"rowbase_i{b}")
                nc.gpsimd.iota(rb_i[:], pattern=[[0, 1]], base=b * P * C,
                               channel_multiplier=C)
                rowbases.append(rb_i)

            scatter_names = set()
            pending = None  # (b, sel, G) awaiting G-stage + scatters

            def emit_gstage_and_scatter(state):
                sb, s_sel, G = state
                # G-stage right AFTER the next block's M-stage on DVE (so sel
                # lands at window+16 not +27), scatters after the gathers on
                # Pool: window = DVE [M_b, G_{b-1}], Pool [g_b, s_{b-1}].
                g_tier = SEL + (sb + 1) * 1000 + 50
                s_tier_a = SEL + (sb + 1) * 1000 + 400
                s_tier_b = SEL + (sb + 1) * 1000 + 400
                Gf = G[:].rearrange("p k l -> p (k l)")
                Gw = g_pool.tile([P, K * cl], F32, tag="Gw")
                gx8 = s_pool.tile([P, 8], F32, tag="gx8")
                gh = []
                gsrc = Gf
                for _ in range(K // 8):
                    gh.append(nc.vector.max(gx8[:], gsrc))
                    gh.append(nc.vector.match_replace(out=Gw[:], in_to_replace=gx8[:],
                                                      in_values=gsrc, imm_value=0.0))
                    gsrc = Gw[:]
                gh.append(nc.vector.tensor_tensor(out=Gw[:], in0=Gf, in1=Gw[:],
                                                  op=mybir.AluOpType.subtract))
                _set_prio(gh, g_tier)
                Gw3 = Gw[:].rearrange("p (k l) -> p k l", l=cl)
                new_ins = []
                if multi:
                    new_ins.append(nc.gpsimd.indirect_dma_start(
                        out=xz_chunks,
                        out_offset=bass.IndirectOffsetOnAxis(ap=s_sel[:, :], axis=0),
                        in_=Gw3[:, :, :],
                        in_offset=None,
                    ))
                else:
                    for k in range(K):
                        new_ins.append(nc.gpsimd.indirect_dma_start(
                            out=xz_chunks,
                            out_offset=bass.IndirectOffsetOnAxis(ap=s_sel[:, k:k + 1], axis=0),
                            in_=Gw3[:, k, :],
                            in_offset=None,
                        ))
                for ins in new_ins:
                    ins_ = getattr(ins, "ins", ins)
                    for dep in list(ins_.sync_dependency_names()):
                        if dep in scatter_names:
                            ins_.try_remove_dependency(dep)
                    scatter_names.add(ins_.name)
                _set_prio(new_ins[:len(new_ins) // 2], s_tier_a)
                _set_prio(new_ins[len(new_ins) // 2:], s_tier_b)

            for b in range(n_blocks):
                r0 = b * P
                M = m_pool.tile([P, C], F32, tag="M")
                stream_h = []
                for s in range(n_sub):
                    c0 = s * sub
                    xt = x_pool.tile([P, sub], F32, tag="xt")
                    h = [nc.sync.dma_start(out=xt[:], in_=x[r0:r0 + P, c0:c0 + sub])]
                    # chunk maxes from RAW x: decouples DVE from the relu
                    h.append(nc.vector.tensor_reduce(
                        out=M[:, s * sub_chunks:(s + 1) * sub_chunks],
                        in_=xt[:].rearrange("p (c l) -> p c l", l=cl),
                        axis=mybir.AxisListType.X,
                        op=mybir.AluOpType.max,
                    ))
                    # relu into a separate tile: the y stream (load -> relu ->
                    # store, all SP/Act) must never wait on DVE's reduce
                    yt = y_pool.tile([P, sub], F32, tag="yt")
                    h.append(nc.scalar.activation(yt[:], xt[:], mybir.ActivationFunctionType.Relu))
                    h.append(nc.scalar.dma_start(out=y[r0:r0 + P, c0:c0 + sub], in_=yt[:]))
                    # unique, monotonically increasing priorities keep the
                    # per-engine tie-break order deterministic
                    for j, hh in enumerate(h):
                        _set_prio([hh], STREAM + (b * n_sub + s) * 8 + j)
                    stream_h += h

                # previous block's G-stage + scatters (tiers computed inside
                # from the pending block's own index)
                if pending is not None:
                    emit_gstage_and_scatter(pending)

                # top-32 chunks: max8 + max_index + match_replace rounds
                m_h = []
                Mw = m_pool.tile([P, C], F32, tag="Mw")
                mx8 = s_pool.tile([P, 8], F32, tag="mx8")
                mi8 = s_pool.tile([P, 8], U32, tag="mi8")
                sel = s_pool.tile([P, K], I32, tag="offs")
                g_h = []
                G = g_pool.tile([P, K, cl], F32, tag="G")
                src = M
                for r in range(K // 8):
                    sl = slice(r * 8, (r + 1) * 8)
                    m_h.append(nc.vector.max(mx8[:], src[:]))
                    m_h.append(nc.vector.max_index(mi8[:], mx8[:], src[:]))
                    if r < K // 8 - 1:
                        m_h.append(nc.vector.match_replace(out=Mw[:], in_to_replace=mx8[:],
                                                           in_values=src[:], imm_value=-1.0))
                        src = Mw
                    # sel = chunk_idx + p*C + b*P*C
                    m_h.append(nc.vector.tensor_tensor(
                        out=sel[:, sl], in0=mi8[:],
                        in1=rowbases[b][:, :1].to_broadcast([P, 8]),
                        op=mybir.AluOpType.add))
                    if not multi:
                        for k in range(r * 8, (r + 1) * 8):
                            g_h.append(nc.gpsimd.indirect_dma_start(
                                out=G[:, k, :], out_offset=None,
                                in_=x_chunks,
                                in_offset=bass.IndirectOffsetOnAxis(ap=sel[:, k:k + 1], axis=0),
                            ))
                if multi:
                    g_h.append(nc.gpsimd.indirect_dma_start(
                        out=G[:, :, :], out_offset=None,
                        in_=x_chunks,
                        in_offset=bass.IndirectOffsetOnAxis(ap=sel[:, :], axis=0),
                    ))
                _set_prio(m_h, SEL + b * 1000)
                _set_prio(g_h, SEL + b * 1000 + 200)
                pending = (b, sel, G)
            if pending is not None:
                emit_gstage_and_scatter(pending)
    nc.finalize()
    return nc


def kernel(x: np.ndarray):
    global LAST_EXEC_TIME_NS, LAST_TRACE_DIR
    x = np.ascontiguousarray(np.asarray(x, dtype=np.float32))
    assert x.shape == (N_ROWS, N_COLS), x.shape
    Rs = N_ROWS // N_CORES

    cfg = {}
    env = os.environ.get("BASS_KCFG")
    if env:
        for kv in env.split(","):
            k, v = kv.split("=")
            cfg[k] = int(v)
    key = tuple(sorted(cfg.items()))
    if key not in _CACHED:
        _CACHED[key] = _build(Rs, N_COLS, **cfg)
    nc = _CACHED[key]

    in_maps = [{"x": x[i * Rs:(i + 1) * Rs]} for i in range(N_CORES)]
    tmpdir = None
    if os.environ.get("BASS_TRACE"):
        import tempfile
        tmpdir = tempfile.mkdtemp(prefix="topk_trace_")
        LAST_TRACE_DIR = tmpdir
    res = run_bass_kernel_spmd(nc, in_maps, core_ids=list(range(N_CORES)),
                               tmpdir=tmpdir)
    LAST_EXEC_TIME_NS = res.exec_time_ns

    y = np.concatenate([np.asarray(res.results[i]["y"]).reshape(Rs, N_COLS)
                        for i in range(N_CORES)], axis=0)
    xz = np.concatenate([np.asarray(res.results[i]["xz"]).reshape(Rs, N_COLS)
                         for i in range(N_CORES)], axis=0)
    return y, xz
